# revision 22
# baseline (speedup 1.0000x reference)
"""EnhancedMultiHeadAttention TRN2 kernel (8 NeuronCores).

Problem (hardcoded shapes): B=4, L=1024, HID=1024, H=16, DH=64, MAX_SEQ=1024.
  q/k/v = x @ W* + b*          (per-head split)
  S = q k^T / sqrt(64) + einsum('bhid,ijd->bhij', q, rel_emb[i-j+1023])
  attn = softmax(S); out = (attn @ v) @ Wo + bo
(The reference's extra renorm attn/(sum+1e-8) is a no-op after softmax.)

Sharding: core c -> batch b = c//2, head group g = c%2 (8 heads each).
Each core computes a partial output x[b]-block @ Wo-rows; host sums the two
partials per batch.

Rel-pos bias trick: qE[i, r'] = q_i . rel_emb[2046-r'] (one matmul against the
flipped table), then bias[i, j] = qE[i, 1023-i+j] is a diagonal re-read of qE
with row stride 1151 inside a [128, 1152] window - done as an SBUF->SBUF DMA
with a hand-built access pattern (flat-element semantics verified on HW),
issued on the GpSimd SWDGE ring (HWDGE rings carry transposes/loads).

The gathered bias is accumulated INTO the score PSUM by an identity matmul
(start=False on the same banks as the q.k matmuls), so no vector-engine add
is needed and exp reads the scores straight out of PSUM.

Attention transpose for the A.V matmul: per-(t,h) bf16 DMA block-transposes
[128, 1024] -> [128, jt, 128] on the SP HWDGE ring, issued right after each
tile's renorm (a big end-of-pair transpose would stall the next pair ~20us).

PSUM is statically partitioned (no shared round-robin ring):
  tag "sps" 2 x [128,1024] (4 banks): proj qt/kt accumulators, score tiles
  tag "qe"  3 x [128,512]  (3 banks): warmup, v, qE chunks, out-proj
  tag "av"  1 x [128,512]  (1 bank):  A.V accumulator (c-sequential)

A.V matmuls of pair p-1 are interleaved between pair p's tile-steps (4 MMs
per step) so the PE queue never drains long enough for HAM to re-throttle
the clock to 1.2 GHz.  All GEMMs are bf16; scores/softmax fp32.  Head pairs
emit their K=64 (qE/S) and M=64 (AV) matmuls adjacently: the auto-derived
tile_position row/col groups (0,*) / (64,*) let the two heads' matmuls
overlap in the array (verified in traces: second of a pair starts ~10ns in).
"""

import ml_dtypes
import numpy as np

import concourse.bass as bass
import concourse.mybir as mybir
import concourse.tile as tile
from concourse.bass_utils import run_bass_kernel_spmd

B, L, HID, H = 4, 1024, 1024, 16
DH = 64
HPC = 8          # heads per core
NPAIR = 4        # head pairs per core
NT = L // 128    # 8 i-tiles
F32 = mybir.dt.float32
BF16 = mybir.dt.bfloat16

_uid = [0]


def _split_multi_waits(nc):
    """Installed walrus accepts 1 sync-wait per instruction (2 for
    EventSemaphore); Tile's tail drain can carry more. Spill extras onto
    EventSemaphore wait-carriers inserted before the offender."""
    for f in nc.m.functions:
        for blk in f.blocks:
            insts = blk.instructions
            idx = 0
            while idx < len(insts):
                inst = insts[idx]
                si = inst.sync_info
                waits = list(si.on_wait) if si and si.on_wait else []
                cap = 2 if type(inst).__name__ == "InstEventSemaphore" else 1
                if len(waits) > cap:
                    si.on_wait = waits[:cap]
                    extra = waits[cap:]
                    carriers = []
                    for k in range(0, len(extra), 2):
                        _uid[0] += 1
                        nop = mybir.InstEventSemaphore(
                            name=f"wait_split_{_uid[0]}", ins=[], outs=[]
                        )
                        nop.engine = inst.engine
                        nop.sync_info = mybir.SyncInfo(
                            on_wait=extra[k:k + 2], on_update=[]
                        )
                        carriers.append(nop)
                    for c in reversed(carriers):
                        insts.insert(idx, c)
                    idx += len(carriers)
                idx += 1


def _ap_with(ap, dims, offset):
    """Return a copy of `ap` with raw [step,count] dims and element offset."""
    c = ap.copy()
    v = c.ap
    assert len(v) == len(dims), (v, dims)
    for i, d in enumerate(dims):
        v[i] = list(d)
    c.ap = v
    c.offset = offset
    return c


def _build_program():
    nc = bass.Bass()

    xT = nc.dram_tensor("xT", (HID, L), BF16, kind="ExternalInput")
    wq = nc.dram_tensor("wq", (HID, 512), BF16, kind="ExternalInput")
    wk = nc.dram_tensor("wk", (HID, 512), BF16, kind="ExternalInput")
    wv = nc.dram_tensor("wv", (HID, 512), BF16, kind="ExternalInput")
    wo = nc.dram_tensor("wo", (512, L), BF16, kind="ExternalInput")
    rt = nc.dram_tensor("rt", (128, 2048), BF16, kind="ExternalInput")
    ident = nc.dram_tensor("ident", (128, 128), BF16, kind="ExternalInput")
    bq = nc.dram_tensor("bq", (512,), F32, kind="ExternalInput")
    bk = nc.dram_tensor("bk", (512,), F32, kind="ExternalInput")
    bv = nc.dram_tensor("bv", (512,), F32, kind="ExternalInput")
    bo = nc.dram_tensor("bo", (L,), F32, kind="ExternalInput")
    out = nc.dram_tensor("out", (L, L), F32, kind="ExternalOutput")

    with tile.TileContext(nc) as tc:
        with tc.tile_pool(name="weights", bufs=1) as wpool, \
             tc.tile_pool(name="proj", bufs=1) as projpool, \
             tc.tile_pool(name="ps", bufs=1, space="PSUM") as ps:

            # ---- resident small tensors ----
            rt_sb = wpool.tile([128, 2048], BF16)
            nc.sync.dma_start(rt_sb[:], rt[:])
            id_sb = wpool.tile([128, 128], BF16)
            nc.sync.dma_start(id_sb[:], ident[:])
            bq_sb = wpool.tile([128, 4], F32)
            nc.sync.dma_start(bq_sb[:], bq[:].rearrange("(t p) -> p t", p=128))
            bk_sb = wpool.tile([128, 4], F32)
            nc.sync.dma_start(bk_sb[:], bk[:].rearrange("(t p) -> p t", p=128))
            # bv replicated across partitions: [1,512] read with partition step 0
            bv_sb = wpool.tile([128, 512], F32)
            nc.sync.dma_start(bv_sb[:], _ap_with(bv[None, :], [[0, 128], [1, 512]], 0))
            bo_sb = wpool.tile([128, 1024], F32)
            nc.sync.dma_start(bo_sb[:], _ap_with(bo[None, :], [[0, 128], [1, 1024]], 0))

            # HAM warm-up: ~4us of junk matmuls on the first-loaded tile so
            # the PE clock is at 8/8 before the projection phase starts
            wu_ps = ps.tile([128, 512], F32, tag="qe", bufs=3, name="wu_ps")
            for wi in range(24):
                nc.tensor.matmul(wu_ps[:], rt_sb[:, 0:128], rt_sb[:, 0:512],
                                 start=(wi == 0), stop=(wi == 23))

            # ---- projections: QT/KT [qdim part, seq], V [seq part, vdim] ----
            qt_sb = [projpool.tile([128, L], BF16, name=f"qt{m}") for m in range(4)]
            kt_sb = [projpool.tile([128, L], BF16, name=f"kt{m}") for m in range(4)]
            v_sb = [projpool.tile([128, 512], BF16, name=f"v{t}") for t in range(NT)]

            with tc.tile_pool(name="p1", bufs=1) as p1:
                # batched loads: one DMA per tensor (32 small DMAs would eat
                # ~13us of descriptor-gen on the SP ring before proj can run)
                xk = p1.tile([128, 8, L], BF16, name="xk")
                wqk = p1.tile([128, 8, 512], BF16, name="wqk")
                wkk = p1.tile([128, 8, 512], BF16, name="wkk")
                wvk = p1.tile([128, 8, 512], BF16, name="wvk")
                xT_r = xT[:].rearrange("(k p) c -> p k c", p=128)
                wq_r = wq[:].rearrange("(k p) c -> p k c", p=128)
                wk_r = wk[:].rearrange("(k p) c -> p k c", p=128)
                wv_r = wv[:].rearrange("(k p) c -> p k c", p=128)
                for lo, hi in ((0, 2), (2, 8)):
                    nc.sync.dma_start(xk[:, lo:hi, :], xT_r[:, lo:hi, :])
                    nc.sync.dma_start(wqk[:, lo:hi, :], wq_r[:, lo:hi, :])
                    nc.sync.dma_start(wkk[:, lo:hi, :], wk_r[:, lo:hi, :])
                    nc.sync.dma_start(wvk[:, lo:hi, :], wv_r[:, lo:hi, :])

                for m in range(4):
                    msl = slice(m * 128, (m + 1) * 128)
                    qt_ps = ps.tile([128, 1024], F32, tag="sps", bufs=2,
                                    name=f"qtp{m}")
                    kt_ps = ps.tile([128, 1024], F32, tag="sps", bufs=2,
                                    name=f"ktp{m}")
                    # stationary wqk[:,k,msl] reused for both c-halves
                    for k in range(8):
                        for c in range(2):
                            csl = slice(c * 512, (c + 1) * 512)
                            nc.tensor.matmul(qt_ps[:, csl], wqk[:, k, msl],
                                             xk[:, k, csl],
                                             start=(k == 0), stop=(k == 7))
                    for k in range(8):
                        for c in range(2):
                            csl = slice(c * 512, (c + 1) * 512)
                            nc.tensor.matmul(kt_ps[:, csl], wkk[:, k, msl],
                                             xk[:, k, csl],
                                             start=(k == 0), stop=(k == 7))
                    nc.any.tensor_scalar_add(qt_sb[m][:], qt_ps[:],
                                             bq_sb[:, m:m + 1])
                    nc.any.tensor_scalar_add(kt_sb[m][:], kt_ps[:],
                                             bk_sb[:, m:m + 1])
                for t in range(NT):
                    tsl = slice(t * 128, (t + 1) * 128)
                    v_ps = ps.tile([128, 512], F32, tag="qe", bufs=3,
                                   name=f"vp{t}")
                    for k in range(8):
                        nc.tensor.matmul(v_ps[:], xk[:, k, tsl], wvk[:, k, :],
                                         start=(k == 0), stop=(k == 7))
                    nc.vector.tensor_tensor(
                        v_sb[t][:], v_ps[:], bv_sb[:], mybir.AluOpType.add)

            wo_sb = wpool.tile([128, 4, L], BF16, name="wo_sb")
            nc.sync.dma_start(wo_sb[:], wo[:].rearrange("(m p) c -> p m c", p=128))

            # ---- attention per head pair ----
            work = tc.alloc_tile_pool(name="work", bufs=3)
            apool = tc.alloc_tile_pool(name="attn", bufs=2)
            opool = tc.alloc_tile_pool(name="outp", bufs=3)
            ctxT_sb = [None] * NPAIR

            def emit_qe(p, t):
                """qE for both heads of pair p, i-tile t.  Head h's matmuls
                use qt/rt partitions [64h, 64h+64) -> auto tile_position
                (64h, 0); same-chunk matmuls of the two heads are adjacent in
                the PE queue so their row groups overlap in the array."""
                qt_p = qt_sb[p]
                w0 = 896 - 128 * t
                isl = slice(t * 128, (t + 1) * 128)
                qe_t = [work.tile([128, 1152], BF16, tag=f"qe{h}", bufs=5,
                                  name=f"qe_sb{h}") for h in range(2)]
                chunks = ((0, 512), (512, 384), (896, 256))
                qe_ps = {}
                for ci, (c0, cw) in enumerate(chunks):
                    for h in range(2):
                        hs = slice(64 * h, 64 * h + 64)
                        qe_ps[ci, h] = ps.tile([128, 512], F32, tag="qe",
                                               bufs=3, name=f"qe{p}{t}{h}{ci}")
                        nc.tensor.matmul(
                            qe_ps[ci, h][:, :cw], qt_p[hs, isl],
                            rt_sb[hs, w0 + c0:w0 + c0 + cw],
                            start=True, stop=True)
                for ci, (c0, cw) in enumerate(chunks):
                    for h in range(2):
                        # split PSUM evacuation: big chunks on DVE, the
                        # 256-wide tail chunks on ACT (keeps both balanced)
                        if ci < 2:
                            nc.vector.tensor_copy(qe_t[h][:, c0:c0 + cw],
                                                  qe_ps[ci, h][:, :cw])
                        else:
                            nc.scalar.copy(qe_t[h][:, c0:c0 + cw],
                                           qe_ps[ci, h][:, :cw])
                # skew gather: bias[q, j] = qe_sb[q, 127 - q + j]
                bias_t = [work.tile([128, L], BF16, tag=f"bias{h}", bufs=6,
                                    name=f"bias_sb{h}") for h in range(2)]
                for h in range(2):
                    nc.gpsimd.dma_start(
                        bias_t[h][:],
                        _ap_with(qe_t[h][:, 0:1024],
                                 [[1151, 128], [1, 1024]], 127))
                return bias_t

            def emit_s(p, t, bias_t, attn_h, sums_h):
                """scores for both heads of pair p, i-tile t: q.k matmuls
                (start=True) then identity matmuls accumulating the gathered
                bias into the same PSUM banks (stop=True); exp reads the
                2-bank score tile straight from PSUM."""
                qt_p = qt_sb[p]
                kt_p = kt_sb[p]
                isl = slice(t * 128, (t + 1) * 128)
                s_ps = [ps.tile([128, 1024], F32, tag="sps", bufs=2,
                                name=f"s_ps{h}") for h in range(2)]
                for c in range(2):
                    csl = slice(c * 512, (c + 1) * 512)
                    for h in range(2):
                        hs = slice(64 * h, 64 * h + 64)
                        nc.tensor.matmul(
                            s_ps[h][:, csl], qt_p[hs, isl], kt_p[hs, csl],
                            start=True, stop=False)
                for c in range(2):
                    csl = slice(c * 512, (c + 1) * 512)
                    for h in range(2):
                        nc.tensor.matmul(
                            s_ps[h][:, csl], id_sb[:], bias_t[h][:, csl],
                            start=False, stop=True)
                for h in range(2):
                    nc.scalar.activation(
                        attn_h[h][:, t, :], s_ps[h][:],
                        mybir.ActivationFunctionType.Exp,
                        accum_out=sums_h[h][:, t:t + 1])
                    recip = work.tile([128, 1], F32, tag=f"recip{h}",
                                      bufs=4, name=f"recip{h}")
                    nc.vector.reciprocal(recip[:], sums_h[h][:, t:t + 1])
                    nc.vector.tensor_scalar_mul(
                        attn_h[h][:, t, :], attn_h[h][:, t, :], recip[:])

            PFD = 3  # bias prefetch distance, in global (p,t) steps

            def av_phase(p, aT, h_outer=False):
                """c-sequential so only ONE PSUM bank is pinned.  h_outer
                orders matmuls so the first half only needs aT[0] (used for
                the final pair, whose transposes drain serially at the end)."""
                ctx = projpool.tile([128, L], BF16, name=f"ctxT{p}")
                for c in range(2):
                    ctx_ps = ps.tile([128, 512], F32, tag="av", bufs=1,
                                     name=f"ctx{p}{c}")
                    order = [(jt, h) for h in range(2) for jt in range(NT)] \
                        if h_outer else \
                        [(jt, h) for jt in range(NT) for h in range(2)]
                    for jt, h in order:
                        nc.tensor.matmul(
                            ctx_ps[64 * h:64 * h + 64, :],
                            v_sb[jt][:, 64 * (2 * p + h):64 * (2 * p + h) + 64],
                            aT[h][:, 4 * c:4 * (c + 1), jt, :],
                            start=(jt == 0), stop=(jt == NT - 1))
                    if c == 0:
                        nc.vector.tensor_copy(ctx[:, 0:512], ctx_ps[:])
                    else:
                        nc.scalar.copy(ctx[:, 512:1024], ctx_ps[:])
                ctxT_sb[p] = ctx

            # global software pipeline over all (p, t) steps: the qE/skew
            # chain runs PFD steps ahead and crosses pair boundaries, so the
            # first score-blocks of pair p+1 never wait on a cold skew chain.
            # Whole-pair transposes go at the pair's end; the av(p-1) block
            # emitted right after keeps the PE busy while they drain.
            seq = [(p, t) for p in range(NPAIR) for t in range(NT)]
            attn_all = {}
            sums_all = {}
            aT_all = {}
            bias_tiles = {}
            for gi in range(len(seq) + PFD):
                if gi < len(seq):
                    p, t = seq[gi]
                    bias_tiles[gi] = emit_qe(p, t)
                if gi >= PFD:
                    p, t = seq[gi - PFD]
                    if t == 0:
                        # allocate here, NOT in the prefetch branch: at this
                        # point every instruction of pair p-1 (exp tail,
                        # transposes) has been emitted, so ring reuse sees
                        # all prior references and can't clobber live tiles
                        attn_all[p] = [
                            apool.tile([128, NT, L], BF16, tag="attn",
                                       bufs=3, name=f"attn{p}_{h}")
                            for h in range(2)]
                        sums_all[p] = [
                            work.tile([128, NT], F32, tag="sums", bufs=4,
                                      name=f"sums{p}_{h}") for h in range(2)]
                        aT_all[p] = [
                            apool.tile([128, NT, NT, 128], BF16, tag="aT",
                                       bufs=3, name=f"aT{p}_{h}")
                            for h in range(2)]
                    emit_s(p, t, bias_tiles.pop(gi - PFD),
                           attn_all[p], sums_all[p])
                    if t == NT - 1:
                        for h in range(2):
                            nc.sync.dma_start(aT_all[p][h][:],
                                              attn_all[p][h][:],
                                              transpose=True)
                        if p > 0:
                            av_phase(p - 1, aT_all.pop(p - 1))
            av_phase(NPAIR - 1, aT_all.pop(NPAIR - 1), h_outer=True)

            # ---- output projection (transpose-mode: ctx[i,hd] @ Wo[hd,o]) ----
            for t in range(NT):
                isl = slice(t * 128, (t + 1) * 128)
                for c in range(2):
                    o_ps = ps.tile([128, 512], F32, tag="qe", bufs=3,
                                   name=f"o{t}{c}")
                    for m in range(4):
                        nc.tensor.matmul(
                            o_ps[:], ctxT_sb[m][:, isl],
                            wo_sb[:, m, c * 512:(c + 1) * 512],
                            start=(m == 0), stop=(m == 3))
                    o_sb = opool.tile([128, 512], F32, tag="osb")
                    nc.vector.tensor_tensor(
                        o_sb[:], o_ps[:], bo_sb[:, c * 512:(c + 1) * 512],
                        mybir.AluOpType.add)
                    nc.sync.dma_start(out[isl, c * 512:(c + 1) * 512], o_sb[:])
            opool.release()
            apool.release()
            work.release()

    _split_multi_waits(nc)
    return nc


_cached = {}


def _get_program():
    if "nc" not in _cached:
        _cached["nc"] = _build_program()
    return _cached["nc"]


def kernel(x, Wq, bq, Wk, bk, Wv, bv, Wo, bo, rel_emb, _timing=None):
    x = np.asarray(x, np.float32)
    Wq = np.asarray(Wq, np.float32)
    Wk = np.asarray(Wk, np.float32)
    Wv = np.asarray(Wv, np.float32)
    Wo = np.asarray(Wo, np.float32)
    bq_ = np.asarray(bq, np.float32)
    bk_ = np.asarray(bk, np.float32)
    bv_ = np.asarray(bv, np.float32)
    bo_ = np.asarray(bo, np.float32)
    rel = np.asarray(rel_emb, np.float32)

    # flipped rel table, transposed, duplicated on both 64-partition halves,
    # padded to 2048 cols
    rt_half = rel[::-1, :].T  # [64, 2047]
    rt_np = np.zeros((128, 2048), ml_dtypes.bfloat16)
    rt_np[0:64, 0:2047] = rt_half.astype(ml_dtypes.bfloat16)
    rt_np[64:128, 0:2047] = rt_half.astype(ml_dtypes.bfloat16)

    bf = ml_dtypes.bfloat16
    id_np = np.eye(128, dtype=bf)
    in_maps = []
    for core in range(8):
        b, g = divmod(core, 2)
        cols = slice(g * 512, (g + 1) * 512)
        in_maps.append({
            "xT": np.ascontiguousarray(x[b].T).astype(bf),
            "wq": np.ascontiguousarray(Wq[:, cols]).astype(bf),
            "wk": (np.ascontiguousarray(Wk[:, cols]) / 8.0).astype(bf),
            "wv": np.ascontiguousarray(Wv[:, cols]).astype(bf),
            "wo": np.ascontiguousarray(Wo[cols, :]).astype(bf),
            "rt": rt_np,
            "ident": id_np,
            "bq": np.ascontiguousarray(bq_[cols]),
            "bk": np.ascontiguousarray(bk_[cols]) / 8.0,
            "bv": np.ascontiguousarray(bv_[cols]),
            "bo": bo_ if g == 0 else np.zeros_like(bo_),
        })

    nc = _get_program()
    kwargs = {}
    if _timing is not None:
        kwargs = dict(trace=True, trace_cores=list(range(8)))
    r = run_bass_kernel_spmd(nc, in_maps, core_ids=list(range(8)), **kwargs)
    if _timing is not None:
        _timing["exec_time_ns"] = r.exec_time_ns
        _timing["mean_exec_time_ns"] = r.mean_exec_time_ns
        _timing["trace"] = r.instructions_and_trace
    outs = [r.results[c]["out"] for c in range(8)]
    return np.stack([outs[2 * b] + outs[2 * b + 1] for b in range(B)], axis=0)
